# revision 26
# baseline (speedup 1.0000x reference)
"""Fused multi-head attention (B=4, L=2048, D=1024, H=16) for 8 Trainium2 cores.

Sharding: core c = 2*b + g handles batch b, head-group g (8 heads).
W_q/W_k sliced+row-permuted (RoPE de-interleave) column-parallel, W_o
row-parallel; host sums the two partial outputs per batch (Megatron-style).

Per-core kernel layout trick: scores are computed TRANSPOSED (S_T[ki, qi])
so that softmax(P) @ V needs no on-chip transpose of P.  Softmax runs
without max-subtraction (logits are bounded ~|s|<6 for this problem's
scale), the /sqrt(hd) and +mask fold into the Exp activation's scale/bias,
and the softmax denominator comes free from a ones-column appended to V.

Optimizations over the first working version (528us -> ~330us):
- Normalization is deferred off the PE critical path: unnormalized o and
  the denominator row are evacuated to SBUF in one [65,1024] PSUM copy,
  the reciprocal uses the fast custom-DVE approx (~1.3us vs 6.5us; input
  hopped to partition 0 first -- custom DVE ops misread on HW when in/out
  partition bases differ), and the broadcast-matmul + scale run one
  attention unit later.  (The old inline [1,1024] reciprocal blocked the
  in-order PE queue for 6.5us per head-half.)
- RoPE as 1 full-tile multiply + 4 shifted-output ops (walrus requires the
  two SBUF inputs of a tensor_tensor to share base partition; outputs are
  free) instead of 12 quarter-tile ops.
- Software pipelining across attention units: PV lags scores by 2, and
  each unit's last PVs + evacuation are deferred into the next unit after
  its first exps are queued, so ACT (the exp engine, ~267us of work) never
  idles at unit boundaries.
- V-projection tiles are emitted inside head 0's t-loop right after each
  exp; chunked x DMA + DMA ordering puts the first exp at ~25us.
- bf16 output (error budget allows it; halves the output DMA).
"""

import sys
from contextlib import ExitStack

import numpy as np

sys.path.insert(0, "/opt/trn_rl_repo")

import ml_dtypes  # noqa: E402

import concourse.bass as bass  # noqa: E402
import concourse.mybir as mybir  # noqa: E402
import concourse.tile as tile  # noqa: E402
from concourse import bacc  # noqa: E402

BF16 = mybir.dt.bfloat16
F32 = mybir.dt.float32
AF = mybir.ActivationFunctionType

B, L, D = 4, 2048, 1024
H, HD = 16, 64
HPC = 8          # heads per core
DH = HPC * HD    # 512 local head dims
NKT = L // 128   # 16 ki tiles
NQB = L // 128   # 16 qi blocks
HALF = 1024      # qi half width


def build_nc(repeats=1):
    nc = bacc.Bacc(
        "TRN2", target_bir_lowering=False, debug=False, enable_asserts=False
    )

    # DRAM I/O (per-core shards, host-prepared layouts)
    xt_d = nc.dram_tensor("xt", [128, 8 * L], BF16, kind="ExternalInput").ap()
    wq_d = nc.dram_tensor("wq", [128, 8 * DH], BF16, kind="ExternalInput").ap()
    wk_d = nc.dram_tensor("wk", [128, 8 * DH], BF16, kind="ExternalInput").ap()
    wv_d = nc.dram_tensor("wv", [128, 8 * DH], BF16, kind="ExternalInput").ap()
    wo_d = nc.dram_tensor("wo", [128, 4 * D], BF16, kind="ExternalInput").ap()
    cos_d = nc.dram_tensor("cosT", [128, L], BF16, kind="ExternalInput").ap()
    sin_d = nc.dram_tensor("sinT", [128, L], BF16, kind="ExternalInput").ap()
    mask_d = nc.dram_tensor("maskT", [128, NKT], F32, kind="ExternalInput").ap()
    out_d = nc.dram_tensor("out", [L, D], BF16, kind="ExternalOutput").ap()

    with tile.TileContext(nc) as tc, ExitStack() as ctx:
        io = ctx.enter_context(tc.tile_pool(name="io", bufs=1))
        tmp = ctx.enter_context(tc.tile_pool(name="tmp", bufs=2))
        esp = ctx.enter_context(tc.tile_pool(name="esp", bufs=7))
        mis = ctx.enter_context(tc.tile_pool(name="mis", bufs=3))
        pp = ctx.enter_context(tc.tile_pool(name="pp", bufs=2, space="PSUM"))

        # ---- load inputs (mask first: tiny and needed by first exp) ----
        maskT = io.tile([128, NKT], F32)
        nc.sync.dma_start(maskT[:], mask_d)
        wq = io.tile([128, 8 * DH], BF16)
        nc.sync.dma_start(wq[:], wq_d)
        xt = io.tile([128, 8 * L], BF16)
        for k in range(8):
            nc.sync.dma_start(
                xt[:, 2048 * k : 2048 * (k + 1)], xt_d[:, 2048 * k : 2048 * (k + 1)]
            )
        wk = io.tile([128, 8 * DH], BF16)
        nc.sync.dma_start(wk[:], wk_d)
        cosT = io.tile([128, L], BF16)
        nc.sync.dma_start(cosT[:], cos_d)
        sinT = io.tile([128, L], BF16)
        nc.sync.dma_start(sinT[:], sin_d)
        wv = io.tile([128, 8 * DH], BF16)
        nc.sync.dma_start(wv[:], wv_d)
        wo = io.tile([128, 4 * D], BF16)
        nc.sync.dma_start(wo[:], wo_d)

        ones64 = io.tile([1, 64], F32)
        nc.vector.memset(ones64[:], 1.0)

        # persistent SBUF activations
        q_sb = [io.tile([128, L], BF16, name=f"q_sb{m}") for m in range(4)]
        k_sb = [io.tile([128, L], BF16, name=f"k_sb{m}") for m in range(4)]
        v_sb = [io.tile([128, HPC * 65], BF16, name=f"v_sb{t}") for t in range(NKT)]
        o_sb = [io.tile([128, L], BF16, name=f"o_sb{m}") for m in range(4)]
        # per (head, half) softmax denominator reciprocal, handed from
        # attn_head to the (lagged) normalize; partition 0 so the broadcast
        # matmul rhs shares base partition 0 with the ones column.
        rec_tiles = {}

        def qk_proj(m, half, w_sb, dst):
            """project tile m (heads 2m, 2m+1), qi-half -> rope -> dst bf16.

            Row layout per 64-row head block: [x0(32) | x1(32)]; cosT/sinT
            rows repeat the 32 rotary frequencies in every 32-block, so one
            full-tile multiply forms x*cos (and x*sin) for every row at once.
            """
            ps = pp.tile([128, HALF], F32, tag="st", name="ps_proj")
            for k in range(8):
                lhsT = w_sb[:, 512 * k + 128 * m : 512 * k + 128 * m + 128]
                for c in range(2):
                    nc.tensor.matmul(
                        ps[:, 512 * c : 512 * (c + 1)],
                        lhsT,
                        xt[:, 2048 * k + HALF * half + 512 * c :][:, :512],
                        start=(k == 0),
                        stop=(k == 7),
                    )
            hs = slice(HALF * half, HALF * (half + 1))
            pre = tmp.tile([128, HALF], BF16, tag="pre")
            nc.vector.tensor_copy(pre[:], ps[:])
            # m1 = x*cos for every row in one full-tile op; the x*sin
            # products are written SHIFTED by 32 partitions (outputs may
            # differ from the common input base partition, inputs may not)
            # so each combine has both inputs on the same partitions.
            m1 = tmp.tile([128, HALF], BF16, tag="m1")
            nc.vector.tensor_mul(m1[:], pre[:], cosT[:, hs])
            t2 = tmp.tile([128, HALF], BF16, tag="m2")
            for hh in range(2):
                o = 64 * hh
                nc.vector.tensor_mul(
                    t2[o : o + 32, :], pre[o + 32 : o + 64, :],
                    sinT[o + 32 : o + 64, hs],
                )
                nc.vector.tensor_sub(
                    dst[o : o + 32, hs], m1[o : o + 32, :], t2[o : o + 32, :]
                )
                nc.vector.tensor_mul(
                    t2[o + 32 : o + 64, :], pre[o : o + 32, :],
                    sinT[o : o + 32, hs],
                )
                nc.vector.tensor_add(
                    dst[o + 32 : o + 64, hs], t2[o + 32 : o + 64, :],
                    m1[o + 32 : o + 64, :],
                )

        def v_proj(kb):
            ps_v = pp.tile([128, DH], F32, tag="st", name="ps_v")
            for k in range(8):
                nc.tensor.matmul(
                    ps_v[:],
                    xt[:, 2048 * k + 128 * kb : 2048 * k + 128 * (kb + 1)],
                    wv[:, 512 * k : 512 * (k + 1)],
                    start=(k == 0),
                    stop=(k == 7),
                )
            vt = v_sb[kb]
            v3 = vt[:].rearrange("p (h c) -> p h c", c=65)
            nc.vector.memset(v3[:, :, 64:65], 1.0)
            nc.vector.tensor_copy(
                v3[:, :, 0:64], ps_v[:].rearrange("p (h c) -> p h c", c=64)
            )

        PVLAG = 2
        tail_hook = {}

        def attn_head(h, half, per_t=None):
            """Scores+exp for t, PV lagged by PVLAG; the last PVLAG PVs and
            the PSUM evacuation are deferred into the NEXT unit (after its
            first exps are queued) so ACT never idles at unit boundaries."""
            m, o = h // 2, 64 * (h % 2)
            p = 2 * h + half
            ot = pp.tile([65, HALF], F32, tag="ot", bufs=2, name="ps_ot")
            es_tiles = {}

            def pv(t):
                es = es_tiles.pop(t)
                for c in range(2):
                    nc.tensor.matmul(
                        ot[:, 512 * c : 512 * (c + 1)],
                        v_sb[t][:, 65 * h : 65 * h + 65],
                        es[:, 512 * c : 512 * (c + 1)],
                        start=(t == 0),
                        stop=(t == NKT - 1),
                    )

            for t in range(NKT):
                st = pp.tile([128, HALF], F32, tag="st", name="ps_st")
                for c in range(2):
                    nc.tensor.matmul(
                        st[:, 512 * c : 512 * (c + 1)],
                        k_sb[m][o : o + 64, 128 * t : 128 * (t + 1)],
                        q_sb[m][o : o + 64, HALF * half + 512 * c :][:, :512],
                        start=True,
                        stop=True,
                    )
                es = esp.tile([128, HALF], BF16, tag="es")
                es_tiles[t] = es
                nc.scalar.activation(
                    es[:], st[:], AF.Exp,
                    bias=maskT[:, t : t + 1], scale=0.125,
                )
                if per_t is not None:
                    per_t(t)
                if t == 2 and tail_hook:
                    tail_hook.pop("f")()
                if t >= PVLAG:
                    pv(t - PVLAG)

            def tail():
                for t in range(NKT - PVLAG, NKT):
                    pv(t)
                # evacuate unnormalized o + denominator in one PSUM read,
                # cast o to bf16, fast-reciprocal the denominator (hopped to
                # partition 0 first: custom DVE ops misread on HW when in/out
                # partition bases differ).
                stage = tmp.tile([65, HALF], F32, tag="stage")
                nc.vector.tensor_copy(stage[:], ot[:])
                nc.vector.tensor_copy(
                    o_sb[m][o : o + 64, HALF * half : HALF * (half + 1)],
                    stage[0:64, :],
                )
                dd = mis.tile([1, HALF], F32, tag="dd", bufs=2)
                nc.vector.tensor_copy(dd[:], stage[64:65, :])
                rec = mis.tile([1, HALF], F32, tag="rec", bufs=3)
                rec_tiles[p] = rec
                nc.vector.reciprocal_approx_fast(rec[:], dd[:])

            tail_hook["f"] = tail

        def flush_tail():
            if tail_hook:
                tail_hook.pop("f")()

        def normalize(h, half):
            """o_sb *= bcast(1/denom); emitted >=1 attention unit after
            attn_head(h, half) so the reciprocal is long done."""
            m, o = h // 2, 64 * (h % 2)
            p = 2 * h + half
            bc = pp.tile([64, HALF], F32, tag="st", name="ps_bc")
            for c in range(2):
                nc.tensor.matmul(
                    bc[:, 512 * c : 512 * (c + 1)],
                    ones64[:],
                    rec_tiles[p][0:1, 512 * c : 512 * (c + 1)],
                    start=True,
                    stop=True,
                )
            sl = o_sb[m][o : o + 64, HALF * half : HALF * (half + 1)]
            nc.vector.tensor_mul(sl, sl, bc[:])

        def outproj_qb(qb):
            ob = mis.tile([128, 1024], BF16, tag="ob", bufs=3)
            for c in range(2):
                po = pp.tile([128, 512], F32, tag="st", name="ps_po")
                for dt_ in range(4):
                    nc.tensor.matmul(
                        po[:],
                        o_sb[dt_][:, 128 * qb : 128 * (qb + 1)],
                        wo[:, D * dt_ + 512 * c : D * dt_ + 512 * (c + 1)],
                        start=(dt_ == 0),
                        stop=(dt_ == 3),
                    )
                nc.vector.tensor_copy(ob[:, 512 * c : 512 * (c + 1)], po[:])
            nc.sync.dma_start(out_d[128 * qb : 128 * (qb + 1), :], ob[:])

        for _rep in range(repeats):
            for half in range(2):
                qk_proj(0, half, wq, q_sb[0])
                qk_proj(0, half, wk, k_sb[0])
            # attention interleaved with remaining projections; v tiles are
            # produced inside head 0's first t-loop, two per iteration right
            # after each exp, so the exp stream starts without waiting for
            # W_v; normalize lags one attention unit behind its head-half.
            def v_bg(t):
                if 2 * t < NKT:
                    v_proj(2 * t)
                    v_proj(2 * t + 1)

            attn_head(0, 0, per_t=v_bg)
            qk_proj(1, 0, wq, q_sb[1]); qk_proj(1, 1, wq, q_sb[1])
            attn_head(0, 1)
            normalize(0, 0)
            qk_proj(1, 0, wk, k_sb[1]); qk_proj(1, 1, wk, k_sb[1])
            attn_head(1, 0)
            normalize(0, 1)
            qk_proj(2, 0, wq, q_sb[2]); qk_proj(2, 1, wq, q_sb[2])
            attn_head(1, 1)
            normalize(1, 0)
            qk_proj(2, 0, wk, k_sb[2]); qk_proj(2, 1, wk, k_sb[2])
            attn_head(2, 0)
            normalize(1, 1)
            qk_proj(3, 0, wq, q_sb[3]); qk_proj(3, 1, wq, q_sb[3])
            attn_head(2, 1)
            normalize(2, 0)
            qk_proj(3, 0, wk, k_sb[3]); qk_proj(3, 1, wk, k_sb[3])
            attn_head(3, 0)
            normalize(2, 1)
            attn_head(3, 1)
            normalize(3, 0)
            for h in range(4, HPC):
                attn_head(h, 0)
                normalize(h - 1, 1)
                attn_head(h, 1)
                normalize(h, 0)
            flush_tail()
            normalize(7, 1)
            for qb in range(NQB):
                outproj_qb(qb)
    nc.compile()
    return nc


def _prep_core_inputs(x, cosT, sinT, mask, W_q, W_k, W_v, W_o, b, g):
    bf = ml_dtypes.bfloat16
    gs = slice(g * DH, (g + 1) * DH)

    # RoPE de-interleave row permutation within the head-group slice
    j = np.arange(64)
    perm64 = np.where(j < 32, 2 * j, 2 * (j - 32) + 1)
    perm = (np.arange(HPC)[:, None] * 64 + perm64[None, :]).reshape(-1) + g * DH

    def wtile(wT):  # [1024, 512] -> [128, 8*512] (k-tile k at cols 512k)
        return np.ascontiguousarray(
            wT.reshape(8, 128, DH).transpose(1, 0, 2).reshape(128, 8 * DH)
        ).astype(bf)

    xt = np.ascontiguousarray(
        x[b].T.reshape(8, 128, L).transpose(1, 0, 2).reshape(128, 8 * L)
    ).astype(bf)
    wq = wtile(W_q[perm].T)
    wk = wtile(W_k[perm].T)
    wv = wtile(W_v[gs].T)
    wo = np.ascontiguousarray(
        W_o[:, gs].T.reshape(4, 128, D).transpose(1, 0, 2).reshape(128, 4 * D)
    ).astype(bf)
    return {
        "xt": xt, "wq": wq, "wk": wk, "wv": wv, "wo": wo,
        "cosT": cosT, "sinT": sinT,
        "maskT": np.ascontiguousarray(mask[b].reshape(NKT, 128).T).astype(
            np.float32
        ),
    }


def make_in_maps(x, freqs_cos, freqs_sin, attention_mask, W_q, W_k, W_v, W_o):
    bf = ml_dtypes.bfloat16
    x = np.asarray(x, np.float32)
    cosT = np.ascontiguousarray(
        np.tile(np.asarray(freqs_cos, np.float32).T, (4, 1))
    ).astype(bf)
    sinT = np.ascontiguousarray(
        np.tile(np.asarray(freqs_sin, np.float32).T, (4, 1))
    ).astype(bf)
    mask = np.asarray(attention_mask, np.float32)
    W_q, W_k = np.asarray(W_q, np.float32), np.asarray(W_k, np.float32)
    W_v, W_o = np.asarray(W_v, np.float32), np.asarray(W_o, np.float32)
    return [
        _prep_core_inputs(x, cosT, sinT, mask, W_q, W_k, W_v, W_o, c // 2, c % 2)
        for c in range(8)
    ]


_CACHE = {}


def kernel(x, freqs_cos, freqs_sin, attention_mask, W_q, W_k, W_v, W_o):
    from concourse.bass_utils import run_bass_kernel_spmd

    if "nc" not in _CACHE:
        _CACHE["nc"] = build_nc()
    nc = _CACHE["nc"]
    in_maps = make_in_maps(
        x, freqs_cos, freqs_sin, attention_mask, W_q, W_k, W_v, W_o
    )
    res = run_bass_kernel_spmd(nc, in_maps, core_ids=list(range(8)))
    outs = [np.asarray(r["out"], dtype=np.float32) for r in res.results]
    full = np.stack([outs[2 * b] + outs[2 * b + 1] for b in range(B)], axis=0)
    return full.astype(np.float32)


if __name__ == "__main__":
    nc = build_nc()
    print("built ok")


# revision 27
# speedup vs baseline: 1.9153x; 1.9153x over previous
"""Fused multi-head attention (B=4, L=2048, D=1024, H=16) for 8 Trainium2 cores.

Sharding: core c = 2*b + g handles batch b, head-group g (8 heads).
W_q/W_k sliced+row-permuted (RoPE de-interleave) column-parallel, W_o
row-parallel; host sums the two partial outputs per batch (Megatron-style).

Per-core kernel layout trick: scores are computed TRANSPOSED (S_T[ki, qi])
so that softmax(P) @ V needs no on-chip transpose of P.  Softmax runs
without max-subtraction (logits are bounded ~|s|<6 for this problem's
scale), the /sqrt(hd) and +mask fold into the Exp activation's scale/bias,
and the softmax denominator comes free from a ones-column appended to V.

Optimizations over the first working version (528us -> ~330us):
- Normalization is deferred off the PE critical path: unnormalized o and
  the denominator row are evacuated to SBUF in one [65,1024] PSUM copy,
  the reciprocal uses the fast custom-DVE approx (~1.3us vs 6.5us; input
  hopped to partition 0 first -- custom DVE ops misread on HW when in/out
  partition bases differ), and the broadcast-matmul + scale run one
  attention unit later.  (The old inline [1,1024] reciprocal blocked the
  in-order PE queue for 6.5us per head-half.)
- RoPE as 1 full-tile multiply + 4 shifted-output ops (walrus requires the
  two SBUF inputs of a tensor_tensor to share base partition; outputs are
  free) instead of 12 quarter-tile ops.
- Software pipelining across attention units: PV lags scores by 2, and
  each unit's last PVs + evacuation are deferred into the next unit after
  its first exps are queued, so ACT (the exp engine, ~267us of work) never
  idles at unit boundaries.
- V-projection tiles are emitted inside head 0's t-loop right after each
  exp; chunked x DMA + DMA ordering puts the first exp at ~25us.
- bf16 output (error budget allows it; halves the output DMA).
"""

import sys
from contextlib import ExitStack

import numpy as np

sys.path.insert(0, "/opt/trn_rl_repo")

import ml_dtypes  # noqa: E402

import concourse.bass as bass  # noqa: E402
import concourse.mybir as mybir  # noqa: E402
import concourse.tile as tile  # noqa: E402
from concourse import bacc  # noqa: E402

BF16 = mybir.dt.bfloat16
F32 = mybir.dt.float32
AF = mybir.ActivationFunctionType

B, L, D = 4, 2048, 1024
H, HD = 16, 64
HPC = 8          # heads per core
DH = HPC * HD    # 512 local head dims
NKT = L // 128   # 16 ki tiles
NQB = L // 128   # 16 qi blocks
HALF = 1024      # qi half width


def build_nc(repeats=1):
    nc = bacc.Bacc(
        "TRN2", target_bir_lowering=False, debug=False, enable_asserts=False
    )

    # DRAM I/O (per-core shards, host-prepared layouts)
    xt_d = nc.dram_tensor("xt", [128, 8 * L], BF16, kind="ExternalInput").ap()
    wq_d = nc.dram_tensor("wq", [128, 8 * DH], BF16, kind="ExternalInput").ap()
    wk_d = nc.dram_tensor("wk", [128, 8 * DH], BF16, kind="ExternalInput").ap()
    wv_d = nc.dram_tensor("wv", [128, 8 * DH], BF16, kind="ExternalInput").ap()
    wo_d = nc.dram_tensor("wo", [128, 4 * D], BF16, kind="ExternalInput").ap()
    cos_d = nc.dram_tensor("cosT", [128, L], BF16, kind="ExternalInput").ap()
    sin_d = nc.dram_tensor("sinT", [128, L], BF16, kind="ExternalInput").ap()
    mask_d = nc.dram_tensor("maskT", [128, NKT], F32, kind="ExternalInput").ap()
    out_d = nc.dram_tensor("out", [L, D], BF16, kind="ExternalOutput").ap()

    with tile.TileContext(nc) as tc, ExitStack() as ctx:
        io = ctx.enter_context(tc.tile_pool(name="io", bufs=1))
        tmp = ctx.enter_context(tc.tile_pool(name="tmp", bufs=2))
        esp = ctx.enter_context(tc.tile_pool(name="esp", bufs=7))
        mis = ctx.enter_context(tc.tile_pool(name="mis", bufs=3))
        pp = ctx.enter_context(tc.tile_pool(name="pp", bufs=3, space="PSUM"))

        # ---- load inputs (mask first: tiny and needed by first exp) ----
        maskT = io.tile([128, NKT], F32)
        nc.sync.dma_start(maskT[:], mask_d)
        wq = io.tile([128, 8 * DH], BF16)
        nc.sync.dma_start(wq[:], wq_d)
        xt = io.tile([128, 8 * L], BF16)
        for k in range(8):
            nc.sync.dma_start(
                xt[:, 2048 * k : 2048 * (k + 1)], xt_d[:, 2048 * k : 2048 * (k + 1)]
            )
        wk = io.tile([128, 8 * DH], BF16)
        nc.sync.dma_start(wk[:], wk_d)
        cosT = io.tile([128, L], BF16)
        nc.sync.dma_start(cosT[:], cos_d)
        sinT = io.tile([128, L], BF16)
        nc.sync.dma_start(sinT[:], sin_d)
        wv = io.tile([128, 8 * DH], BF16)
        nc.sync.dma_start(wv[:], wv_d)
        wo = io.tile([128, 4 * D], BF16)
        nc.sync.dma_start(wo[:], wo_d)

        ones64 = io.tile([1, 64], F32)
        nc.vector.memset(ones64[:], 1.0)

        # persistent SBUF activations
        q_sb = [io.tile([128, L], BF16, name=f"q_sb{m}") for m in range(4)]
        k_sb = [io.tile([128, L], BF16, name=f"k_sb{m}") for m in range(4)]
        v_sb = [io.tile([128, HPC * 65], BF16, name=f"v_sb{t}") for t in range(NKT)]
        o_sb = [io.tile([128, L], BF16, name=f"o_sb{m}") for m in range(4)]
        # per (head, half) softmax denominator reciprocal, handed from
        # attn_head to the (lagged) normalize; partition 0 so the broadcast
        # matmul rhs shares base partition 0 with the ones column.
        rec_tiles = {}

        def qk_proj(m, half, w_sb, dst):
            """project tile m (heads 2m, 2m+1), qi-half -> rope -> dst bf16.

            Row layout per 64-row head block: [x0(32) | x1(32)]; cosT/sinT
            rows repeat the 32 rotary frequencies in every 32-block, so one
            full-tile multiply forms x*cos (and x*sin) for every row at once.
            """
            ps = pp.tile([128, HALF], F32, tag="st", name="ps_proj")
            for k in range(8):
                lhsT = w_sb[:, 512 * k + 128 * m : 512 * k + 128 * m + 128]
                for c in range(2):
                    nc.tensor.matmul(
                        ps[:, 512 * c : 512 * (c + 1)],
                        lhsT,
                        xt[:, 2048 * k + HALF * half + 512 * c :][:, :512],
                        start=(k == 0),
                        stop=(k == 7),
                    )
            hs = slice(HALF * half, HALF * (half + 1))
            pre = tmp.tile([128, HALF], BF16, tag="pre")
            nc.vector.tensor_copy(pre[:], ps[:])
            # m1 = x*cos for every row in one full-tile op; the x*sin
            # products are written SHIFTED by 32 partitions (outputs may
            # differ from the common input base partition, inputs may not)
            # so each combine has both inputs on the same partitions.
            m1 = tmp.tile([128, HALF], BF16, tag="m1")
            nc.vector.tensor_mul(m1[:], pre[:], cosT[:, hs])
            t2 = tmp.tile([128, HALF], BF16, tag="m2")
            for hh in range(2):
                o = 64 * hh
                nc.vector.tensor_mul(
                    t2[o : o + 32, :], pre[o + 32 : o + 64, :],
                    sinT[o + 32 : o + 64, hs],
                )
                nc.vector.tensor_sub(
                    dst[o : o + 32, hs], m1[o : o + 32, :], t2[o : o + 32, :]
                )
                nc.vector.tensor_mul(
                    t2[o + 32 : o + 64, :], pre[o : o + 32, :],
                    sinT[o : o + 32, hs],
                )
                nc.vector.tensor_add(
                    dst[o + 32 : o + 64, hs], t2[o + 32 : o + 64, :],
                    m1[o + 32 : o + 64, :],
                )

        def v_proj(kb):
            ps_v = pp.tile([128, DH], F32, tag="st", name="ps_v")
            for k in range(8):
                nc.tensor.matmul(
                    ps_v[:],
                    xt[:, 2048 * k + 128 * kb : 2048 * k + 128 * (kb + 1)],
                    wv[:, 512 * k : 512 * (k + 1)],
                    start=(k == 0),
                    stop=(k == 7),
                )
            vt = v_sb[kb]
            v3 = vt[:].rearrange("p (h c) -> p h c", c=65)
            nc.vector.memset(v3[:, :, 64:65], 1.0)
            nc.vector.tensor_copy(
                v3[:, :, 0:64], ps_v[:].rearrange("p (h c) -> p h c", c=64)
            )

        PVLAG = 2
        tail_hook = {}

        def attn_head(h, half, per_t=None):
            """Scores+exp for t, PV lagged by PVLAG; the last PVLAG PVs and
            the PSUM evacuation are deferred into the NEXT unit (after its
            first exps are queued) so ACT never idles at unit boundaries."""
            m, o = h // 2, 64 * (h % 2)
            p = 2 * h + half
            ot = pp.tile([65, HALF], F32, tag="ot", bufs=1, name="ps_ot")
            es_tiles = {}

            def pv(t):
                es = es_tiles.pop(t)
                for c in range(2):
                    nc.tensor.matmul(
                        ot[:, 512 * c : 512 * (c + 1)],
                        v_sb[t][:, 65 * h : 65 * h + 65],
                        es[:, 512 * c : 512 * (c + 1)],
                        start=(t == 0),
                        stop=(t == NKT - 1),
                    )

            for t in range(NKT):
                st = pp.tile([128, HALF], F32, tag="st", name="ps_st")
                for c in range(2):
                    nc.tensor.matmul(
                        st[:, 512 * c : 512 * (c + 1)],
                        k_sb[m][o : o + 64, 128 * t : 128 * (t + 1)],
                        q_sb[m][o : o + 64, HALF * half + 512 * c :][:, :512],
                        start=True,
                        stop=True,
                    )
                es = esp.tile([128, HALF], BF16, tag="es")
                es_tiles[t] = es
                nc.scalar.activation(
                    es[:], st[:], AF.Exp,
                    bias=maskT[:, t : t + 1], scale=0.125,
                )
                if per_t is not None:
                    per_t(t)
                if t == 2 and tail_hook:
                    tail_hook.pop("f")()
                if t >= PVLAG:
                    pv(t - PVLAG)

            def tail():
                for t in range(NKT - PVLAG, NKT):
                    pv(t)
                # evacuate unnormalized o + denominator in one PSUM read,
                # cast o to bf16, fast-reciprocal the denominator (hopped to
                # partition 0 first: custom DVE ops misread on HW when in/out
                # partition bases differ).
                stage = tmp.tile([65, HALF], F32, tag="stage")
                nc.vector.tensor_copy(stage[:], ot[:])
                nc.vector.tensor_copy(
                    o_sb[m][o : o + 64, HALF * half : HALF * (half + 1)],
                    stage[0:64, :],
                )
                dd = mis.tile([1, HALF], F32, tag="dd", bufs=2)
                nc.vector.tensor_copy(dd[:], stage[64:65, :])
                rec = mis.tile([1, HALF], F32, tag="rec", bufs=3)
                rec_tiles[p] = rec
                nc.vector.reciprocal_approx_fast(rec[:], dd[:])

            tail_hook["f"] = tail

        def flush_tail():
            if tail_hook:
                tail_hook.pop("f")()

        def normalize(h, half):
            """o_sb *= bcast(1/denom); emitted >=1 attention unit after
            attn_head(h, half) so the reciprocal is long done."""
            m, o = h // 2, 64 * (h % 2)
            p = 2 * h + half
            bc = pp.tile([64, HALF], F32, tag="st", name="ps_bc")
            for c in range(2):
                nc.tensor.matmul(
                    bc[:, 512 * c : 512 * (c + 1)],
                    ones64[:],
                    rec_tiles[p][0:1, 512 * c : 512 * (c + 1)],
                    start=True,
                    stop=True,
                )
            sl = o_sb[m][o : o + 64, HALF * half : HALF * (half + 1)]
            nc.vector.tensor_mul(sl, sl, bc[:])

        def outproj_qb(qb):
            ob = mis.tile([128, 1024], BF16, tag="ob", bufs=3)
            for c in range(2):
                po = pp.tile([128, 512], F32, tag="st", name="ps_po")
                for dt_ in range(4):
                    nc.tensor.matmul(
                        po[:],
                        o_sb[dt_][:, 128 * qb : 128 * (qb + 1)],
                        wo[:, D * dt_ + 512 * c : D * dt_ + 512 * (c + 1)],
                        start=(dt_ == 0),
                        stop=(dt_ == 3),
                    )
                nc.vector.tensor_copy(ob[:, 512 * c : 512 * (c + 1)], po[:])
            nc.sync.dma_start(out_d[128 * qb : 128 * (qb + 1), :], ob[:])

        for _rep in range(repeats):
            for half in range(2):
                qk_proj(0, half, wq, q_sb[0])
                qk_proj(0, half, wk, k_sb[0])
            # attention interleaved with remaining projections; v tiles are
            # produced inside head 0's first t-loop, two per iteration right
            # after each exp, so the exp stream starts without waiting for
            # W_v; normalize lags one attention unit behind its head-half.
            def v_bg(t):
                if 2 * t < NKT:
                    v_proj(2 * t)
                    v_proj(2 * t + 1)

            attn_head(0, 0, per_t=v_bg)
            qk_proj(1, 0, wq, q_sb[1]); qk_proj(1, 1, wq, q_sb[1])
            attn_head(0, 1)
            normalize(0, 0)
            qk_proj(1, 0, wk, k_sb[1]); qk_proj(1, 1, wk, k_sb[1])
            attn_head(1, 0)
            normalize(0, 1)
            qk_proj(2, 0, wq, q_sb[2]); qk_proj(2, 1, wq, q_sb[2])
            attn_head(1, 1)
            normalize(1, 0)
            qk_proj(2, 0, wk, k_sb[2]); qk_proj(2, 1, wk, k_sb[2])
            attn_head(2, 0)
            normalize(1, 1)
            qk_proj(3, 0, wq, q_sb[3]); qk_proj(3, 1, wq, q_sb[3])
            attn_head(2, 1)
            normalize(2, 0)
            qk_proj(3, 0, wk, k_sb[3]); qk_proj(3, 1, wk, k_sb[3])
            attn_head(3, 0)
            normalize(2, 1)
            attn_head(3, 1)
            normalize(3, 0)
            for h in range(4, HPC):
                attn_head(h, 0)
                normalize(h - 1, 1)
                attn_head(h, 1)
                normalize(h, 0)
            flush_tail()
            normalize(7, 1)
            for qb in range(NQB):
                outproj_qb(qb)
    nc.compile()
    return nc


def _prep_core_inputs(x, cosT, sinT, mask, W_q, W_k, W_v, W_o, b, g):
    bf = ml_dtypes.bfloat16
    gs = slice(g * DH, (g + 1) * DH)

    # RoPE de-interleave row permutation within the head-group slice
    j = np.arange(64)
    perm64 = np.where(j < 32, 2 * j, 2 * (j - 32) + 1)
    perm = (np.arange(HPC)[:, None] * 64 + perm64[None, :]).reshape(-1) + g * DH

    def wtile(wT):  # [1024, 512] -> [128, 8*512] (k-tile k at cols 512k)
        return np.ascontiguousarray(
            wT.reshape(8, 128, DH).transpose(1, 0, 2).reshape(128, 8 * DH)
        ).astype(bf)

    xt = np.ascontiguousarray(
        x[b].T.reshape(8, 128, L).transpose(1, 0, 2).reshape(128, 8 * L)
    ).astype(bf)
    wq = wtile(W_q[perm].T)
    wk = wtile(W_k[perm].T)
    wv = wtile(W_v[gs].T)
    wo = np.ascontiguousarray(
        W_o[:, gs].T.reshape(4, 128, D).transpose(1, 0, 2).reshape(128, 4 * D)
    ).astype(bf)
    return {
        "xt": xt, "wq": wq, "wk": wk, "wv": wv, "wo": wo,
        "cosT": cosT, "sinT": sinT,
        "maskT": np.ascontiguousarray(mask[b].reshape(NKT, 128).T).astype(
            np.float32
        ),
    }


def make_in_maps(x, freqs_cos, freqs_sin, attention_mask, W_q, W_k, W_v, W_o):
    bf = ml_dtypes.bfloat16
    x = np.asarray(x, np.float32)
    cosT = np.ascontiguousarray(
        np.tile(np.asarray(freqs_cos, np.float32).T, (4, 1))
    ).astype(bf)
    sinT = np.ascontiguousarray(
        np.tile(np.asarray(freqs_sin, np.float32).T, (4, 1))
    ).astype(bf)
    mask = np.asarray(attention_mask, np.float32)
    W_q, W_k = np.asarray(W_q, np.float32), np.asarray(W_k, np.float32)
    W_v, W_o = np.asarray(W_v, np.float32), np.asarray(W_o, np.float32)
    return [
        _prep_core_inputs(x, cosT, sinT, mask, W_q, W_k, W_v, W_o, c // 2, c % 2)
        for c in range(8)
    ]


_CACHE = {}


def kernel(x, freqs_cos, freqs_sin, attention_mask, W_q, W_k, W_v, W_o):
    from concourse.bass_utils import run_bass_kernel_spmd

    if "nc" not in _CACHE:
        _CACHE["nc"] = build_nc()
    nc = _CACHE["nc"]
    in_maps = make_in_maps(
        x, freqs_cos, freqs_sin, attention_mask, W_q, W_k, W_v, W_o
    )
    res = run_bass_kernel_spmd(nc, in_maps, core_ids=list(range(8)))
    outs = [np.asarray(r["out"], dtype=np.float32) for r in res.results]
    full = np.stack([outs[2 * b] + outs[2 * b + 1] for b in range(B)], axis=0)
    return full.astype(np.float32)


if __name__ == "__main__":
    nc = build_nc()
    print("built ok")


# revision 28
# speedup vs baseline: 1.9371x; 1.0114x over previous
"""Fused multi-head attention (B=4, L=2048, D=1024, H=16) for 8 Trainium2 cores.

Sharding: core c = 2*b + g handles batch b, head-group g (8 heads).
W_q/W_k sliced+row-permuted (RoPE de-interleave) column-parallel, W_o
row-parallel; host sums the two partial outputs per batch (Megatron-style).

Per-core kernel layout trick: scores are computed TRANSPOSED (S_T[ki, qi])
so that softmax(P) @ V needs no on-chip transpose of P.  Softmax runs
without max-subtraction (logits are bounded ~|s|<6 for this problem's
scale), the /sqrt(hd) and +mask fold into the Exp activation's scale/bias,
and the softmax denominator comes free from a ones-column appended to V.

Optimizations over the first working version (528us -> ~330us):
- Normalization is deferred off the PE critical path: unnormalized o and
  the denominator row are evacuated to SBUF in one [65,1024] PSUM copy,
  the reciprocal uses the fast custom-DVE approx (~1.3us vs 6.5us; input
  hopped to partition 0 first -- custom DVE ops misread on HW when in/out
  partition bases differ), and the broadcast-matmul + scale run one
  attention unit later.  (The old inline [1,1024] reciprocal blocked the
  in-order PE queue for 6.5us per head-half.)
- RoPE as 1 full-tile multiply + 4 shifted-output ops (walrus requires the
  two SBUF inputs of a tensor_tensor to share base partition; outputs are
  free) instead of 12 quarter-tile ops.
- Software pipelining across attention units: PV lags scores by 2, and
  each unit's last PVs + evacuation are deferred into the next unit after
  its first exps are queued, so ACT (the exp engine, ~267us of work) never
  idles at unit boundaries.
- V-projection tiles are emitted inside head 0's t-loop right after each
  exp; chunked x DMA + DMA ordering puts the first exp at ~25us.
- bf16 output (error budget allows it; halves the output DMA).
"""

import sys
from contextlib import ExitStack

import numpy as np

sys.path.insert(0, "/opt/trn_rl_repo")

import ml_dtypes  # noqa: E402

import concourse.bass as bass  # noqa: E402
import concourse.mybir as mybir  # noqa: E402
import concourse.tile as tile  # noqa: E402
from concourse import bacc  # noqa: E402

BF16 = mybir.dt.bfloat16
F32 = mybir.dt.float32
AF = mybir.ActivationFunctionType

B, L, D = 4, 2048, 1024
H, HD = 16, 64
HPC = 8          # heads per core
DH = HPC * HD    # 512 local head dims
NKT = L // 128   # 16 ki tiles
NQB = L // 128   # 16 qi blocks
HALF = 1024      # qi half width


def build_nc(repeats=1):
    nc = bacc.Bacc(
        "TRN2", target_bir_lowering=False, debug=False, enable_asserts=False
    )

    # DRAM I/O (per-core shards, host-prepared layouts)
    xt_d = nc.dram_tensor("xt", [128, 8 * L], BF16, kind="ExternalInput").ap()
    wq_d = nc.dram_tensor("wq", [128, 8 * DH], BF16, kind="ExternalInput").ap()
    wk_d = nc.dram_tensor("wk", [128, 8 * DH], BF16, kind="ExternalInput").ap()
    wv_d = nc.dram_tensor("wv", [128, 8 * DH], BF16, kind="ExternalInput").ap()
    wo_d = nc.dram_tensor("wo", [128, 4 * D], BF16, kind="ExternalInput").ap()
    cos_d = nc.dram_tensor("cosT", [128, L], BF16, kind="ExternalInput").ap()
    sin_d = nc.dram_tensor("sinT", [128, L], BF16, kind="ExternalInput").ap()
    mask_d = nc.dram_tensor("maskT", [128, NKT], F32, kind="ExternalInput").ap()
    out_d = nc.dram_tensor("out", [L, D], BF16, kind="ExternalOutput").ap()

    with tile.TileContext(nc) as tc, ExitStack() as ctx:
        io = ctx.enter_context(tc.tile_pool(name="io", bufs=1))
        tmp = ctx.enter_context(tc.tile_pool(name="tmp", bufs=2))
        esp = ctx.enter_context(tc.tile_pool(name="esp", bufs=8))
        mis = ctx.enter_context(tc.tile_pool(name="mis", bufs=3))
        pp = ctx.enter_context(tc.tile_pool(name="pp", bufs=3, space="PSUM"))

        # ---- load inputs (mask first: tiny and needed by first exp) ----
        maskT = io.tile([128, NKT], F32)
        nc.sync.dma_start(maskT[:], mask_d)
        wq = io.tile([128, 8 * DH], BF16)
        nc.sync.dma_start(wq[:], wq_d)
        xt = io.tile([128, 8 * L], BF16)
        for k in range(8):
            nc.sync.dma_start(
                xt[:, 2048 * k : 2048 * (k + 1)], xt_d[:, 2048 * k : 2048 * (k + 1)]
            )
        wk = io.tile([128, 8 * DH], BF16)
        nc.sync.dma_start(wk[:], wk_d)
        cosT = io.tile([128, L], BF16)
        nc.sync.dma_start(cosT[:], cos_d)
        sinT = io.tile([128, L], BF16)
        nc.sync.dma_start(sinT[:], sin_d)
        wv = io.tile([128, 8 * DH], BF16)
        nc.sync.dma_start(wv[:], wv_d)
        wo = io.tile([128, 4 * D], BF16)
        nc.sync.dma_start(wo[:], wo_d)

        ones64 = io.tile([1, 64], F32)
        nc.vector.memset(ones64[:], 1.0)

        # persistent SBUF activations
        q_sb = [io.tile([128, L], BF16, name=f"q_sb{m}") for m in range(4)]
        k_sb = [io.tile([128, L], BF16, name=f"k_sb{m}") for m in range(4)]
        v_sb = [io.tile([128, HPC * 65], BF16, name=f"v_sb{t}") for t in range(NKT)]
        o_sb = [io.tile([128, L], BF16, name=f"o_sb{m}") for m in range(4)]
        # per (head, half) softmax denominator reciprocal, handed from
        # attn_head to the (lagged) normalize; partition 0 so the broadcast
        # matmul rhs shares base partition 0 with the ones column.
        rec_tiles = {}

        def qk_proj(m, half, w_sb, dst):
            """project tile m (heads 2m, 2m+1), qi-half -> rope -> dst bf16.

            Row layout per 64-row head block: [x0(32) | x1(32)]; cosT/sinT
            rows repeat the 32 rotary frequencies in every 32-block, so one
            full-tile multiply forms x*cos (and x*sin) for every row at once.
            """
            ps = pp.tile([128, HALF], F32, tag="st", name="ps_proj")
            for k in range(8):
                lhsT = w_sb[:, 512 * k + 128 * m : 512 * k + 128 * m + 128]
                for c in range(2):
                    nc.tensor.matmul(
                        ps[:, 512 * c : 512 * (c + 1)],
                        lhsT,
                        xt[:, 2048 * k + HALF * half + 512 * c :][:, :512],
                        start=(k == 0),
                        stop=(k == 7),
                    )
            hs = slice(HALF * half, HALF * (half + 1))
            pre = tmp.tile([128, HALF], BF16, tag="pre")
            nc.vector.tensor_copy(pre[:], ps[:])
            # m1 = x*cos for every row in one full-tile op; the x*sin
            # products are written SHIFTED by 32 partitions (outputs may
            # differ from the common input base partition, inputs may not)
            # so each combine has both inputs on the same partitions.
            m1 = tmp.tile([128, HALF], BF16, tag="m1")
            nc.vector.tensor_mul(m1[:], pre[:], cosT[:, hs])
            t2 = tmp.tile([128, HALF], BF16, tag="m2")
            for hh in range(2):
                o = 64 * hh
                nc.vector.tensor_mul(
                    t2[o : o + 32, :], pre[o + 32 : o + 64, :],
                    sinT[o + 32 : o + 64, hs],
                )
                nc.vector.tensor_sub(
                    dst[o : o + 32, hs], m1[o : o + 32, :], t2[o : o + 32, :]
                )
                nc.vector.tensor_mul(
                    t2[o + 32 : o + 64, :], pre[o : o + 32, :],
                    sinT[o : o + 32, hs],
                )
                nc.vector.tensor_add(
                    dst[o + 32 : o + 64, hs], t2[o + 32 : o + 64, :],
                    m1[o + 32 : o + 64, :],
                )

        def v_proj(kb):
            ps_v = pp.tile([128, DH], F32, tag="st", name="ps_v")
            for k in range(8):
                nc.tensor.matmul(
                    ps_v[:],
                    xt[:, 2048 * k + 128 * kb : 2048 * k + 128 * (kb + 1)],
                    wv[:, 512 * k : 512 * (k + 1)],
                    start=(k == 0),
                    stop=(k == 7),
                )
            vt = v_sb[kb]
            v3 = vt[:].rearrange("p (h c) -> p h c", c=65)
            nc.vector.memset(v3[:, :, 64:65], 1.0)
            nc.vector.tensor_copy(
                v3[:, :, 0:64], ps_v[:].rearrange("p (h c) -> p h c", c=64)
            )

        PVLAG = 2
        tail_hook = {}

        def attn_head(h, half, per_t=None):
            """Scores+exp for t, PV lagged by PVLAG; the last PVLAG PVs and
            the PSUM evacuation are deferred into the NEXT unit (after its
            first exps are queued) so ACT never idles at unit boundaries."""
            m, o = h // 2, 64 * (h % 2)
            p = 2 * h + half
            ot = pp.tile([65, HALF], F32, tag="ot", bufs=1, name="ps_ot")
            es_tiles = {}

            def pv(t):
                es = es_tiles.pop(t)
                for c in range(2):
                    nc.tensor.matmul(
                        ot[:, 512 * c : 512 * (c + 1)],
                        v_sb[t][:, 65 * h : 65 * h + 65],
                        es[:, 512 * c : 512 * (c + 1)],
                        start=(t == 0),
                        stop=(t == NKT - 1),
                    )

            for t in range(NKT):
                st = pp.tile([128, HALF], F32, tag="st", name="ps_st")
                for c in range(2):
                    nc.tensor.matmul(
                        st[:, 512 * c : 512 * (c + 1)],
                        k_sb[m][o : o + 64, 128 * t : 128 * (t + 1)],
                        q_sb[m][o : o + 64, HALF * half + 512 * c :][:, :512],
                        start=True,
                        stop=True,
                    )
                es = esp.tile([128, HALF], BF16, tag="es")
                es_tiles[t] = es
                nc.scalar.activation(
                    es[:], st[:], AF.Exp,
                    bias=maskT[:, t : t + 1], scale=0.125,
                )
                if per_t is not None:
                    per_t(t)
                if t == 2 and tail_hook:
                    tail_hook.pop("f")()
                if t >= PVLAG:
                    pv(t - PVLAG)

            def tail():
                for t in range(NKT - PVLAG, NKT):
                    pv(t)
                # evacuate unnormalized o + denominator in one PSUM read,
                # cast o to bf16, fast-reciprocal the denominator (hopped to
                # partition 0 first: custom DVE ops misread on HW when in/out
                # partition bases differ).
                stage = tmp.tile([65, HALF], F32, tag="stage")
                nc.vector.tensor_copy(stage[:], ot[:])
                nc.vector.tensor_copy(
                    o_sb[m][o : o + 64, HALF * half : HALF * (half + 1)],
                    stage[0:64, :],
                )
                dd = mis.tile([1, HALF], F32, tag="dd", bufs=2)
                nc.vector.tensor_copy(dd[:], stage[64:65, :])
                rec = mis.tile([1, HALF], F32, tag="rec", bufs=3)
                rec_tiles[p] = rec
                nc.vector.reciprocal_approx_fast(rec[:], dd[:])

            tail_hook["f"] = tail

        def flush_tail():
            if tail_hook:
                tail_hook.pop("f")()

        def normalize(h, half):
            """o_sb *= bcast(1/denom); emitted >=1 attention unit after
            attn_head(h, half) so the reciprocal is long done."""
            m, o = h // 2, 64 * (h % 2)
            p = 2 * h + half
            bc = pp.tile([64, HALF], F32, tag="st", name="ps_bc")
            for c in range(2):
                nc.tensor.matmul(
                    bc[:, 512 * c : 512 * (c + 1)],
                    ones64[:],
                    rec_tiles[p][0:1, 512 * c : 512 * (c + 1)],
                    start=True,
                    stop=True,
                )
            sl = o_sb[m][o : o + 64, HALF * half : HALF * (half + 1)]
            nc.vector.tensor_mul(sl, sl, bc[:])

        def outproj_qb(qb):
            ob = mis.tile([128, 1024], BF16, tag="ob", bufs=3)
            for c in range(2):
                po = pp.tile([128, 512], F32, tag="st", name="ps_po")
                for dt_ in range(4):
                    nc.tensor.matmul(
                        po[:],
                        o_sb[dt_][:, 128 * qb : 128 * (qb + 1)],
                        wo[:, D * dt_ + 512 * c : D * dt_ + 512 * (c + 1)],
                        start=(dt_ == 0),
                        stop=(dt_ == 3),
                    )
                nc.vector.tensor_copy(ob[:, 512 * c : 512 * (c + 1)], po[:])
            nc.sync.dma_start(out_d[128 * qb : 128 * (qb + 1), :], ob[:])

        for _rep in range(repeats):
            for half in range(2):
                qk_proj(0, half, wq, q_sb[0])
                qk_proj(0, half, wk, k_sb[0])
            # attention interleaved with remaining projections; v tiles are
            # produced inside head 0's first t-loop, two per iteration right
            # after each exp, so the exp stream starts without waiting for
            # W_v; normalize lags one attention unit behind its head-half.
            def v_bg(t):
                if 2 * t < NKT:
                    v_proj(2 * t)
                    v_proj(2 * t + 1)

            attn_head(0, 0, per_t=v_bg)
            qk_proj(1, 0, wq, q_sb[1]); qk_proj(1, 1, wq, q_sb[1])
            attn_head(0, 1)
            normalize(0, 0)
            qk_proj(1, 0, wk, k_sb[1]); qk_proj(1, 1, wk, k_sb[1])
            attn_head(1, 0)
            normalize(0, 1)
            qk_proj(2, 0, wq, q_sb[2]); qk_proj(2, 1, wq, q_sb[2])
            attn_head(1, 1)
            normalize(1, 0)
            qk_proj(2, 0, wk, k_sb[2]); qk_proj(2, 1, wk, k_sb[2])
            attn_head(2, 0)
            normalize(1, 1)
            qk_proj(3, 0, wq, q_sb[3]); qk_proj(3, 1, wq, q_sb[3])
            attn_head(2, 1)
            normalize(2, 0)
            qk_proj(3, 0, wk, k_sb[3]); qk_proj(3, 1, wk, k_sb[3])
            attn_head(3, 0)
            normalize(2, 1)
            attn_head(3, 1)
            normalize(3, 0)
            for h in range(4, HPC):
                attn_head(h, 0)
                normalize(h - 1, 1)
                attn_head(h, 1)
                normalize(h, 0)
            flush_tail()
            normalize(7, 1)
            for qb in range(NQB):
                outproj_qb(qb)
    nc.compile()
    return nc


def _prep_core_inputs(x, cosT, sinT, mask, W_q, W_k, W_v, W_o, b, g):
    bf = ml_dtypes.bfloat16
    gs = slice(g * DH, (g + 1) * DH)

    # RoPE de-interleave row permutation within the head-group slice
    j = np.arange(64)
    perm64 = np.where(j < 32, 2 * j, 2 * (j - 32) + 1)
    perm = (np.arange(HPC)[:, None] * 64 + perm64[None, :]).reshape(-1) + g * DH

    def wtile(wT):  # [1024, 512] -> [128, 8*512] (k-tile k at cols 512k)
        return np.ascontiguousarray(
            wT.reshape(8, 128, DH).transpose(1, 0, 2).reshape(128, 8 * DH)
        ).astype(bf)

    xt = np.ascontiguousarray(
        x[b].T.reshape(8, 128, L).transpose(1, 0, 2).reshape(128, 8 * L)
    ).astype(bf)
    wq = wtile(W_q[perm].T)
    wk = wtile(W_k[perm].T)
    wv = wtile(W_v[gs].T)
    wo = np.ascontiguousarray(
        W_o[:, gs].T.reshape(4, 128, D).transpose(1, 0, 2).reshape(128, 4 * D)
    ).astype(bf)
    return {
        "xt": xt, "wq": wq, "wk": wk, "wv": wv, "wo": wo,
        "cosT": cosT, "sinT": sinT,
        "maskT": np.ascontiguousarray(mask[b].reshape(NKT, 128).T).astype(
            np.float32
        ),
    }


def make_in_maps(x, freqs_cos, freqs_sin, attention_mask, W_q, W_k, W_v, W_o):
    bf = ml_dtypes.bfloat16
    x = np.asarray(x, np.float32)
    cosT = np.ascontiguousarray(
        np.tile(np.asarray(freqs_cos, np.float32).T, (4, 1))
    ).astype(bf)
    sinT = np.ascontiguousarray(
        np.tile(np.asarray(freqs_sin, np.float32).T, (4, 1))
    ).astype(bf)
    mask = np.asarray(attention_mask, np.float32)
    W_q, W_k = np.asarray(W_q, np.float32), np.asarray(W_k, np.float32)
    W_v, W_o = np.asarray(W_v, np.float32), np.asarray(W_o, np.float32)
    return [
        _prep_core_inputs(x, cosT, sinT, mask, W_q, W_k, W_v, W_o, c // 2, c % 2)
        for c in range(8)
    ]


_CACHE = {}


def kernel(x, freqs_cos, freqs_sin, attention_mask, W_q, W_k, W_v, W_o):
    from concourse.bass_utils import run_bass_kernel_spmd

    if "nc" not in _CACHE:
        _CACHE["nc"] = build_nc()
    nc = _CACHE["nc"]
    in_maps = make_in_maps(
        x, freqs_cos, freqs_sin, attention_mask, W_q, W_k, W_v, W_o
    )
    res = run_bass_kernel_spmd(nc, in_maps, core_ids=list(range(8)))
    outs = [np.asarray(r["out"], dtype=np.float32) for r in res.results]
    full = np.stack([outs[2 * b] + outs[2 * b + 1] for b in range(B)], axis=0)
    return full.astype(np.float32)


if __name__ == "__main__":
    nc = build_nc()
    print("built ok")
